# revision 8
# baseline (speedup 1.0000x reference)
"""Trainium2 Bass kernel for nn_MemoryModule_17145509445673 (scatter_memory).

Math (reference):
    f_emb   = batch_embedding * batch_frequency[:, None]              # [T, D]
    contrib = addr_t * f_emb[:, :, None]          (addr_t = [T, D, S])
    write   = segment_sum(contrib, segment_ids)                       # [B, D, S]
    mem     = memory_matrix + write
    basic_read[t, d] = sum_s addr[t, d, s] * mem[seg[t], d, s]
    cm_info = basic_read / batch_embedding ; cm_readhead = min_d cm_info

Strategy: shard the 4096 items across 8 cores (segment_ids are sorted, so
this is nearly segment-aligned).  Each core streams its 52 MB slice of
batch_address from HBM exactly once, in 5 slot-pairs of 1024 (2x512):
  - the A DMA casts fp32 -> bf16 in the SDMA datapath (SWDGE), so matmuls
    run at the PE's 1 cycle/row bf16 rate
  - PE computes the partial segment-sum w (lhsT = one-hot * f_emb),
    folding memory_matrix in via an identity matmul (nonzero on core 0 only)
  - an 8-core AllReduce completes w -> mem for the pair
  - PE re-expands mem per item (one-hot gather matmul), DVE multiplies the
    resident A tile into the gather PSUM tile in place, ACT row-reduces it
    into per-pair partial dots
"""

import sys

if "/opt/trn_rl_repo" not in sys.path:
    sys.path.insert(0, "/opt/trn_rl_repo")

import numpy as np
import ml_dtypes

from concourse import bacc, tile, mybir, bass_utils

DEP, TOTAL, SLOT, NSEG = 5, 4096, 5120, 16
NCORES = 8
TLOC = TOTAL // NCORES          # 512 items per core
KT = TLOC // 128                # 4 item tiles of 128
SC = 512                        # matmul slot granularity (PSUM bank)
PW = 2 * SC                     # 1024: phase-B pair width
NPAIR = SLOT // PW              # 5
F32 = mybir.dt.float32
BF16 = mybir.dt.bfloat16

_CACHED = None


def _build():
    nc = bacc.Bacc("TRN2", target_bir_lowering=False, debug=False,
                   enable_asserts=False, num_devices=NCORES)

    a_d = nc.dram_tensor("a", [DEP, TLOC, SLOT], F32, kind="ExternalInput").ap()
    f_d = nc.dram_tensor("f", [128, KT, DEP, NSEG], BF16, kind="ExternalInput").ap()
    oh_d = nc.dram_tensor("oh", [NSEG, KT, 128], BF16, kind="ExternalInput").ap()
    mm_d = nc.dram_tensor("mm", [NPAIR, NSEG, DEP, PW], BF16, kind="ExternalInput").ap()
    eye_d = nc.dram_tensor("eye", [NSEG, NSEG], BF16, kind="ExternalInput").ap()
    emb_d = nc.dram_tensor("emb", [128, KT, DEP], F32, kind="ExternalInput").ap()

    br_d = nc.dram_tensor("br", [TLOC, DEP], F32, kind="ExternalOutput").ap()
    ci_d = nc.dram_tensor("ci", [TLOC, DEP], F32, kind="ExternalOutput").ap()
    crh_d = nc.dram_tensor("crh", [TLOC, 1], F32, kind="ExternalOutput").ap()

    with tile.TileContext(nc) as tc:
        with (
            tc.tile_pool(name="const", bufs=1) as const,
            tc.tile_pool(name="a_pool", bufs=2 * KT) as a_pool,
            tc.tile_pool(name="wsb", bufs=2) as wsb_pool,
            tc.tile_pool(name="mem", bufs=2) as mem_pool,
            tc.tile_pool(name="mmc", bufs=2) as mmc_pool,
            tc.tile_pool(name="prod", bufs=3) as prod_pool,
            tc.tile_pool(name="wps", bufs=2, space="PSUM") as wps_pool,
            tc.tile_pool(name="gps", bufs=3, space="PSUM") as gps_pool,
            tc.tile_pool(name="dram", bufs=4, space="DRAM") as dram_pool,
        ):
            f_sb = const.tile([128, KT, DEP, NSEG], BF16)
            oh_sb = const.tile([NSEG, KT, 128], BF16)
            eye_sb = const.tile([NSEG, NSEG], BF16)
            emb_sb = const.tile([128, KT, DEP], F32)
            r_all = const.tile([128, KT, DEP, NPAIR], F32)
            nc.sync.dma_start(f_sb[:], f_d[:])
            nc.sync.dma_start(oh_sb[:], oh_d[:])
            nc.sync.dma_start(eye_sb[:], eye_d[:])
            nc.sync.dma_start(emb_sb[:], emb_d[:])

            def load_pair(i):
                # A tiles: fp32 in HBM -> bf16 in SBUF (SWDGE cast), one DMA
                # per (item tile, half) so compute starts on the first half
                tiles = []
                for k in range(KT):
                    at = a_pool.tile([128, DEP, PW], BF16, tag="a_tile")
                    tiles.append(at)
                for h in range(2):
                    for k in range(KT):
                        src = a_d[:, k * 128:(k + 1) * 128,
                                  i * PW + h * SC: i * PW + (h + 1) * SC]
                        nc.gpsimd.dma_start(tiles[k][:, :, h * SC:(h + 1) * SC],
                                            src.rearrange("d p s -> p d s"))
                return tiles

            pending = {0: load_pair(0)}
            for i in range(NPAIR):
                # issue the NEXT pair's A loads before anything that depends
                # on this pair's collective, so the gpsimd DMA queue is not
                # head-of-line blocked behind CC(i)
                if i + 1 < NPAIR:
                    pending[i + 1] = load_pair(i + 1)
                a_tiles = pending.pop(i)

                mm_sb = mmc_pool.tile([NSEG, DEP, PW], BF16)
                nc.sync.dma_start(mm_sb[:], mm_d[i])

                # phase A: per-core partial segment sums (+ memory_matrix on core 0)
                w_sb = wsb_pool.tile([NSEG, DEP, PW], F32)
                for d in range(DEP):
                    for h in range(2):
                        sl = slice(h * SC, (h + 1) * SC)
                        w_ps = wps_pool.tile([NSEG, SC], F32)
                        nc.tensor.matmul(w_ps[:], eye_sb[:], mm_sb[:, d, sl],
                                         start=True, stop=False)
                        for k in range(KT):
                            nc.tensor.matmul(w_ps[:], f_sb[:, k, d, :],
                                             a_tiles[k][:, d, sl],
                                             start=False, stop=(k == KT - 1))
                        nc.vector.tensor_copy(w_sb[:, d, sl], w_ps[:])

                # complete the segment sum across cores
                cc_in = dram_pool.tile([NSEG, DEP, PW], F32, tag="cc_in")
                cc_out = dram_pool.tile([NSEG, DEP, PW], F32, tag="cc_out")
                nc.sync.dma_start(cc_in[:], w_sb[:])
                nc.gpsimd.collective_compute(
                    "AllReduce", mybir.AluOpType.add,
                    replica_groups=[list(range(NCORES))],
                    ins=[cc_in.opt()], outs=[cc_out.opt()],
                )
                mem_sb = mem_pool.tile([NSEG, DEP, PW], BF16)
                nc.gpsimd.dma_start(mem_sb[:], cc_out[:])  # SWDGE cast f32->bf16

                # phase B: per-item dot with its segment's memory row
                for k in range(KT):
                    for d in range(DEP):
                        g_ps = gps_pool.tile([128, PW], F32)
                        for h in range(2):
                            sl = slice(h * SC, (h + 1) * SC)
                            nc.tensor.matmul(g_ps[:, sl], oh_sb[:, k, :],
                                             mem_sb[:, d, sl],
                                             start=True, stop=True)
                        prod = prod_pool.tile([128, PW], F32)
                        nc.vector.tensor_tensor(
                            out=prod[:], in0=a_tiles[k][:, d, :],
                            in1=g_ps[:], op=mybir.AluOpType.mult)
                        nc.scalar.activation(
                            out=prod[:], in_=prod[:],
                            func=mybir.ActivationFunctionType.Copy,
                            accum_out=r_all[:, k, d, i:i + 1])

            # epilogue: br = sum over pairs; ci = br / emb; crh = min_d ci
            br_sb = const.tile([128, KT, DEP], F32)
            nc.vector.tensor_reduce(out=br_sb[:], in_=r_all[:],
                                    axis=mybir.AxisListType.X,
                                    op=mybir.AluOpType.add)
            recip = const.tile([128, KT, DEP], F32)
            nc.vector.reciprocal(recip[:], emb_sb[:])
            ci_sb = const.tile([128, KT, DEP], F32)
            nc.vector.tensor_tensor(out=ci_sb[:], in0=br_sb[:], in1=recip[:],
                                    op=mybir.AluOpType.mult)
            crh_sb = const.tile([128, KT], F32)
            nc.vector.tensor_reduce(out=crh_sb[:], in_=ci_sb[:],
                                    axis=mybir.AxisListType.X,
                                    op=mybir.AluOpType.min)
            for k in range(KT):
                nc.sync.dma_start(br_d[k * 128:(k + 1) * 128, :], br_sb[:, k, :])
                nc.sync.dma_start(ci_d[k * 128:(k + 1) * 128, :], ci_sb[:, k, :])
                nc.sync.dma_start(crh_d[k * 128:(k + 1) * 128, :], crh_sb[:, k:k + 1])

    nc.compile()
    return nc


def _get_nc():
    global _CACHED
    if _CACHED is None:
        _CACHED = _build()
    return _CACHED


def kernel(batch_address, batch_embedding, batch_frequency, memory_matrix,
           segment_ids, _run_kwargs=None):
    addr = np.ascontiguousarray(np.asarray(batch_address, dtype=np.float32))
    emb = np.asarray(batch_embedding, dtype=np.float32)
    freq = np.asarray(batch_frequency, dtype=np.float32)
    mm = np.asarray(memory_matrix, dtype=np.float32)
    seg = np.asarray(segment_ids, dtype=np.int32)

    femb = emb * freq[:, None]                                   # [T, D]
    onehot = (seg[:, None] == np.arange(NSEG)[None, :]).astype(np.float32)
    F = femb[:, :, None] * onehot[:, None, :]                    # [T, D, B]
    eye = np.eye(NSEG, dtype=ml_dtypes.bfloat16)
    mm_chunks = np.ascontiguousarray(
        mm.reshape(NSEG, DEP, NPAIR, PW).transpose(2, 0, 1, 3)).astype(
            ml_dtypes.bfloat16)                                  # [5,16,5,1024]
    mm_zero = np.zeros_like(mm_chunks)

    in_maps = []
    for c in range(NCORES):
        sl = slice(c * TLOC, (c + 1) * TLOC)
        f_c = np.ascontiguousarray(
            F[sl].reshape(KT, 128, DEP, NSEG).transpose(1, 0, 2, 3)).astype(
                ml_dtypes.bfloat16)
        oh_c = np.ascontiguousarray(
            onehot[sl].reshape(KT, 128, NSEG).transpose(2, 0, 1)).astype(
                ml_dtypes.bfloat16)
        emb_c = np.ascontiguousarray(
            emb[sl].reshape(KT, 128, DEP).transpose(1, 0, 2))
        in_maps.append({
            "a": np.ascontiguousarray(addr[:, sl, :]),
            "f": f_c,
            "oh": oh_c,
            "mm": mm_chunks if c == 0 else mm_zero,
            "eye": eye,
            "emb": emb_c,
        })

    nc = _get_nc()
    res = bass_utils.run_bass_kernel_spmd(
        nc, in_maps, core_ids=list(range(NCORES)), **(_run_kwargs or {}))

    br = np.concatenate([res.results[c]["br"] for c in range(NCORES)], axis=0)
    ci = np.concatenate([res.results[c]["ci"] for c in range(NCORES)], axis=0)
    crh = np.concatenate([res.results[c]["crh"] for c in range(NCORES)], axis=0)
    kernel.last_results = res
    return crh, ci, br


# revision 9
# speedup vs baseline: 1.1239x; 1.1239x over previous
"""Trainium2 Bass kernel for nn_MemoryModule_17145509445673 (scatter_memory).

Math (reference):
    f_emb   = batch_embedding * batch_frequency[:, None]              # [T, D]
    contrib = addr_t * f_emb[:, :, None]          (addr_t = [T, D, S])
    write   = segment_sum(contrib, segment_ids)                       # [B, D, S]
    mem     = memory_matrix + write
    basic_read[t, d] = sum_s addr[t, d, s] * mem[seg[t], d, s]
    cm_info = basic_read / batch_embedding ; cm_readhead = min_d cm_info

Strategy: shard the 4096 items across 8 cores (segment_ids are sorted, so
this is nearly segment-aligned).  Each core streams its 52 MB slice of
batch_address from HBM exactly once, in 5 slot-pairs of 1024 (2x512):
  - the A DMA casts fp32 -> bf16 in the SDMA datapath (SWDGE), so matmuls
    run at the PE's 1 cycle/row bf16 rate
  - PE computes the partial segment-sum w (lhsT = one-hot * f_emb),
    folding memory_matrix in via an identity matmul (nonzero on core 0 only)
  - an 8-core AllReduce completes w -> mem for the pair
  - PE re-expands mem per item (one-hot gather matmul), DVE multiplies the
    resident A tile into the gather PSUM tile in place, ACT row-reduces it
    into per-pair partial dots
"""

import sys

if "/opt/trn_rl_repo" not in sys.path:
    sys.path.insert(0, "/opt/trn_rl_repo")

import numpy as np
import ml_dtypes

from concourse import bacc, tile, mybir, bass_utils

DEP, TOTAL, SLOT, NSEG = 5, 4096, 5120, 16
NCORES = 8
TLOC = TOTAL // NCORES          # 512 items per core
KT = TLOC // 128                # 4 item tiles of 128
SC = 512                        # matmul slot granularity (PSUM bank)
PW = 2 * SC                     # 1024: phase-B pair width
NPAIR = SLOT // PW              # 5
F32 = mybir.dt.float32
BF16 = mybir.dt.bfloat16

_CACHED = None


def _build():
    nc = bacc.Bacc("TRN2", target_bir_lowering=False, debug=False,
                   enable_asserts=False, num_devices=NCORES)

    a_d = nc.dram_tensor("a", [DEP, TLOC, SLOT], F32, kind="ExternalInput").ap()
    f_d = nc.dram_tensor("f", [128, KT, DEP, NSEG], BF16, kind="ExternalInput").ap()
    oh_d = nc.dram_tensor("oh", [NSEG, KT, 128], BF16, kind="ExternalInput").ap()
    mm_d = nc.dram_tensor("mm", [NPAIR, NSEG, DEP, PW], BF16, kind="ExternalInput").ap()
    eye_d = nc.dram_tensor("eye", [NSEG, NSEG], BF16, kind="ExternalInput").ap()
    emb_d = nc.dram_tensor("emb", [128, KT, DEP], F32, kind="ExternalInput").ap()

    br_d = nc.dram_tensor("br", [TLOC, DEP], F32, kind="ExternalOutput").ap()
    ci_d = nc.dram_tensor("ci", [TLOC, DEP], F32, kind="ExternalOutput").ap()
    crh_d = nc.dram_tensor("crh", [TLOC, 1], F32, kind="ExternalOutput").ap()

    with tile.TileContext(nc) as tc:
        with (
            tc.tile_pool(name="const", bufs=1) as const,
            tc.tile_pool(name="a_pool", bufs=11) as a_pool,
            tc.tile_pool(name="wsb", bufs=2) as wsb_pool,
            tc.tile_pool(name="mem", bufs=2) as mem_pool,
            tc.tile_pool(name="mmc", bufs=1) as mmc_pool,
            tc.tile_pool(name="prod", bufs=3) as prod_pool,
            tc.tile_pool(name="wps", bufs=2, space="PSUM") as wps_pool,
            tc.tile_pool(name="gps", bufs=3, space="PSUM") as gps_pool,
            tc.tile_pool(name="dram", bufs=4, space="DRAM") as dram_pool,
        ):
            f_sb = const.tile([128, KT, DEP, NSEG], BF16)
            oh_sb = const.tile([NSEG, KT, 128], BF16)
            eye_sb = const.tile([NSEG, NSEG], BF16)
            emb_sb = const.tile([128, KT, DEP], F32)
            r_all = const.tile([128, KT, DEP, NPAIR], F32)
            nc.sync.dma_start(f_sb[:], f_d[:])
            nc.sync.dma_start(oh_sb[:], oh_d[:])
            nc.sync.dma_start(eye_sb[:], eye_d[:])
            nc.sync.dma_start(emb_sb[:], emb_d[:])

            def load_pair(i):
                # A tiles: fp32 in HBM -> bf16 in SBUF (SWDGE cast), one DMA
                # per (item tile, half) so compute starts on the first half
                tiles = []
                for k in range(KT):
                    at = a_pool.tile([128, DEP, PW], BF16, tag="a_tile")
                    tiles.append(at)
                for h in range(2):
                    for k in range(KT):
                        src = a_d[:, k * 128:(k + 1) * 128,
                                  i * PW + h * SC: i * PW + (h + 1) * SC]
                        nc.gpsimd.dma_start(tiles[k][:, :, h * SC:(h + 1) * SC],
                                            src.rearrange("d p s -> p d s"))
                return tiles

            pending = {0: load_pair(0)}
            for i in range(NPAIR):
                # issue the NEXT pair's A loads before anything that depends
                # on this pair's collective, so the gpsimd DMA queue is not
                # head-of-line blocked behind CC(i)
                if i + 1 < NPAIR:
                    pending[i + 1] = load_pair(i + 1)
                a_tiles = pending.pop(i)

                mm_sb = mmc_pool.tile([NSEG, DEP, PW], BF16)
                nc.sync.dma_start(mm_sb[:], mm_d[i])

                # phase A: per-core partial segment sums (+ memory_matrix on core 0)
                w_sb = wsb_pool.tile([NSEG, DEP, PW], BF16)
                for d in range(DEP):
                    for h in range(2):
                        sl = slice(h * SC, (h + 1) * SC)
                        w_ps = wps_pool.tile([NSEG, SC], F32)
                        nc.tensor.matmul(w_ps[:], eye_sb[:], mm_sb[:, d, sl],
                                         start=True, stop=False)
                        for k in range(KT):
                            nc.tensor.matmul(w_ps[:], f_sb[:, k, d, :],
                                             a_tiles[k][:, d, sl],
                                             start=False, stop=(k == KT - 1))
                        nc.scalar.copy(w_sb[:, d, sl], w_ps[:])

                # complete the segment sum across cores
                cc_in = dram_pool.tile([NSEG, DEP, PW], BF16, tag="cc_in")
                cc_out = dram_pool.tile([NSEG, DEP, PW], BF16, tag="cc_out")
                nc.sync.dma_start(cc_in[:], w_sb[:])
                nc.gpsimd.collective_compute(
                    "AllReduce", mybir.AluOpType.add,
                    replica_groups=[list(range(NCORES))],
                    ins=[cc_in.opt()], outs=[cc_out.opt()],
                )
                mem_sb = mem_pool.tile([NSEG, DEP, PW], BF16)
                nc.sync.dma_start(mem_sb[:], cc_out[:])

                # phase B: per-item dot with its segment's memory row
                for k in range(KT):
                    for d in range(DEP):
                        g_ps = gps_pool.tile([128, PW], F32)
                        for h in range(2):
                            sl = slice(h * SC, (h + 1) * SC)
                            nc.tensor.matmul(g_ps[:, sl], oh_sb[:, k, :],
                                             mem_sb[:, d, sl],
                                             start=True, stop=True)
                        prod = prod_pool.tile([128, PW], BF16)
                        nc.vector.tensor_tensor(
                            out=prod[:], in0=a_tiles[k][:, d, :],
                            in1=g_ps[:], op=mybir.AluOpType.mult)
                        nc.scalar.activation(
                            out=prod[:], in_=prod[:],
                            func=mybir.ActivationFunctionType.Copy,
                            accum_out=r_all[:, k, d, i:i + 1])

            # epilogue: br = sum over pairs; ci = br / emb; crh = min_d ci
            br_sb = const.tile([128, KT, DEP], F32)
            nc.vector.tensor_reduce(out=br_sb[:], in_=r_all[:],
                                    axis=mybir.AxisListType.X,
                                    op=mybir.AluOpType.add)
            recip = const.tile([128, KT, DEP], F32)
            nc.vector.reciprocal(recip[:], emb_sb[:])
            ci_sb = const.tile([128, KT, DEP], F32)
            nc.vector.tensor_tensor(out=ci_sb[:], in0=br_sb[:], in1=recip[:],
                                    op=mybir.AluOpType.mult)
            crh_sb = const.tile([128, KT], F32)
            nc.vector.tensor_reduce(out=crh_sb[:], in_=ci_sb[:],
                                    axis=mybir.AxisListType.X,
                                    op=mybir.AluOpType.min)
            for k in range(KT):
                nc.sync.dma_start(br_d[k * 128:(k + 1) * 128, :], br_sb[:, k, :])
                nc.sync.dma_start(ci_d[k * 128:(k + 1) * 128, :], ci_sb[:, k, :])
                nc.sync.dma_start(crh_d[k * 128:(k + 1) * 128, :], crh_sb[:, k:k + 1])

    nc.compile()
    return nc


def _get_nc():
    global _CACHED
    if _CACHED is None:
        _CACHED = _build()
    return _CACHED


def kernel(batch_address, batch_embedding, batch_frequency, memory_matrix,
           segment_ids, _run_kwargs=None):
    addr = np.ascontiguousarray(np.asarray(batch_address, dtype=np.float32))
    emb = np.asarray(batch_embedding, dtype=np.float32)
    freq = np.asarray(batch_frequency, dtype=np.float32)
    mm = np.asarray(memory_matrix, dtype=np.float32)
    seg = np.asarray(segment_ids, dtype=np.int32)

    femb = emb * freq[:, None]                                   # [T, D]
    onehot = (seg[:, None] == np.arange(NSEG)[None, :]).astype(np.float32)
    F = femb[:, :, None] * onehot[:, None, :]                    # [T, D, B]
    eye = np.eye(NSEG, dtype=ml_dtypes.bfloat16)
    mm_chunks = np.ascontiguousarray(
        mm.reshape(NSEG, DEP, NPAIR, PW).transpose(2, 0, 1, 3)).astype(
            ml_dtypes.bfloat16)                                  # [5,16,5,1024]
    mm_zero = np.zeros_like(mm_chunks)

    in_maps = []
    for c in range(NCORES):
        sl = slice(c * TLOC, (c + 1) * TLOC)
        f_c = np.ascontiguousarray(
            F[sl].reshape(KT, 128, DEP, NSEG).transpose(1, 0, 2, 3)).astype(
                ml_dtypes.bfloat16)
        oh_c = np.ascontiguousarray(
            onehot[sl].reshape(KT, 128, NSEG).transpose(2, 0, 1)).astype(
                ml_dtypes.bfloat16)
        emb_c = np.ascontiguousarray(
            emb[sl].reshape(KT, 128, DEP).transpose(1, 0, 2))
        in_maps.append({
            "a": np.ascontiguousarray(addr[:, sl, :]),
            "f": f_c,
            "oh": oh_c,
            "mm": mm_chunks if c == 0 else mm_zero,
            "eye": eye,
            "emb": emb_c,
        })

    nc = _get_nc()
    res = bass_utils.run_bass_kernel_spmd(
        nc, in_maps, core_ids=list(range(NCORES)), **(_run_kwargs or {}))

    br = np.concatenate([res.results[c]["br"] for c in range(NCORES)], axis=0)
    ci = np.concatenate([res.results[c]["ci"] for c in range(NCORES)], axis=0)
    crh = np.concatenate([res.results[c]["crh"] for c in range(NCORES)], axis=0)
    kernel.last_results = res
    return crh, ci, br


# revision 10
# speedup vs baseline: 1.1830x; 1.0526x over previous
"""Trainium2 Bass kernel for nn_MemoryModule_17145509445673 (scatter_memory).

Math (reference):
    f_emb   = batch_embedding * batch_frequency[:, None]              # [T, D]
    contrib = addr_t * f_emb[:, :, None]          (addr_t = [T, D, S])
    write   = segment_sum(contrib, segment_ids)                       # [B, D, S]
    mem     = memory_matrix + write
    basic_read[t, d] = sum_s addr[t, d, s] * mem[seg[t], d, s]
    cm_info = basic_read / batch_embedding ; cm_readhead = min_d cm_info

Strategy: shard the 4096 items across 8 cores (segment_ids are sorted, so
this is nearly segment-aligned).  Each core streams its 52 MB slice of
batch_address from HBM exactly once, in 5 slot-pairs of 1024 (2x512):
  - the A DMA casts fp32 -> bf16 in the SDMA datapath (SWDGE), so matmuls
    run at the PE's 1 cycle/row bf16 rate
  - PE computes the partial segment-sum w (lhsT = one-hot * f_emb),
    folding memory_matrix in via an identity matmul (nonzero on core 0 only)
  - an 8-core AllReduce completes w -> mem for the pair
  - PE re-expands mem per item (one-hot gather matmul), DVE multiplies the
    resident A tile into the gather PSUM tile in place, ACT row-reduces it
    into per-pair partial dots
"""

import sys

if "/opt/trn_rl_repo" not in sys.path:
    sys.path.insert(0, "/opt/trn_rl_repo")

import numpy as np
import ml_dtypes

from concourse import bacc, tile, mybir, bass_utils

DEP, TOTAL, SLOT, NSEG = 5, 4096, 5120, 16
NCORES = 8
TLOC = TOTAL // NCORES          # 512 items per core
KT = TLOC // 128                # 4 item tiles of 128
SC = 512                        # matmul slot granularity (PSUM bank)
PW = 2 * SC                     # 1024: phase-B pair width
NPAIR = SLOT // PW              # 5
F32 = mybir.dt.float32
BF16 = mybir.dt.bfloat16

_CACHED = None


def _build():
    nc = bacc.Bacc("TRN2", target_bir_lowering=False, debug=False,
                   enable_asserts=False, num_devices=NCORES)

    a_d = nc.dram_tensor("a", [DEP, TLOC, SLOT], BF16, kind="ExternalInput").ap()
    f_d = nc.dram_tensor("f", [128, KT, DEP, NSEG], BF16, kind="ExternalInput").ap()
    oh_d = nc.dram_tensor("oh", [NSEG, KT, 128], BF16, kind="ExternalInput").ap()
    mm_d = nc.dram_tensor("mm", [NPAIR, NSEG, DEP, PW], BF16, kind="ExternalInput").ap()
    eye_d = nc.dram_tensor("eye", [NSEG, NSEG], BF16, kind="ExternalInput").ap()
    emb_d = nc.dram_tensor("emb", [128, KT, DEP], F32, kind="ExternalInput").ap()

    br_d = nc.dram_tensor("br", [TLOC, DEP], F32, kind="ExternalOutput").ap()
    ci_d = nc.dram_tensor("ci", [TLOC, DEP], F32, kind="ExternalOutput").ap()
    crh_d = nc.dram_tensor("crh", [TLOC, 1], F32, kind="ExternalOutput").ap()

    with tile.TileContext(nc) as tc:
        with (
            tc.tile_pool(name="const", bufs=1) as const,
            tc.tile_pool(name="a_pool", bufs=11) as a_pool,
            tc.tile_pool(name="wsb", bufs=2) as wsb_pool,
            tc.tile_pool(name="mem", bufs=2) as mem_pool,
            tc.tile_pool(name="mmc", bufs=1) as mmc_pool,
            tc.tile_pool(name="prod", bufs=3) as prod_pool,
            tc.tile_pool(name="wps", bufs=2, space="PSUM") as wps_pool,
            tc.tile_pool(name="gps", bufs=3, space="PSUM") as gps_pool,
            tc.tile_pool(name="dram", bufs=4, space="DRAM") as dram_pool,
        ):
            f_sb = const.tile([128, KT, DEP, NSEG], BF16)
            oh_sb = const.tile([NSEG, KT, 128], BF16)
            eye_sb = const.tile([NSEG, NSEG], BF16)
            emb_sb = const.tile([128, KT, DEP], F32)
            r_all = const.tile([128, KT, DEP, NPAIR], F32)
            nc.sync.dma_start(f_sb[:], f_d[:])
            nc.sync.dma_start(oh_sb[:], oh_d[:])
            nc.sync.dma_start(eye_sb[:], eye_d[:])
            nc.sync.dma_start(emb_sb[:], emb_d[:])

            def load_pair(i):
                # A tiles: fp32 in HBM -> bf16 in SBUF (SWDGE cast), one DMA
                # per (item tile, half) so compute starts on the first half
                tiles = []
                for k in range(KT):
                    at = a_pool.tile([128, DEP, PW], BF16, tag="a_tile")
                    tiles.append(at)
                for h in range(2):
                    for k in range(KT):
                        src = a_d[:, k * 128:(k + 1) * 128,
                                  i * PW + h * SC: i * PW + (h + 1) * SC]
                        nc.sync.dma_start(tiles[k][:, :, h * SC:(h + 1) * SC],
                                          src.rearrange("d p s -> p d s"))
                return tiles

            pending = {0: load_pair(0)}
            for i in range(NPAIR):
                # issue the NEXT pair's A loads before anything that depends
                # on this pair's collective, so the gpsimd DMA queue is not
                # head-of-line blocked behind CC(i)
                if i + 1 < NPAIR:
                    pending[i + 1] = load_pair(i + 1)
                a_tiles = pending.pop(i)

                mm_sb = mmc_pool.tile([NSEG, DEP, PW], BF16)
                nc.sync.dma_start(mm_sb[:], mm_d[i])

                # phase A: per-core partial segment sums (+ memory_matrix on core 0)
                w_sb = wsb_pool.tile([NSEG, DEP, PW], BF16)
                for d in range(DEP):
                    for h in range(2):
                        sl = slice(h * SC, (h + 1) * SC)
                        w_ps = wps_pool.tile([NSEG, SC], F32)
                        nc.tensor.matmul(w_ps[:], eye_sb[:], mm_sb[:, d, sl],
                                         start=True, stop=False)
                        for k in range(KT):
                            nc.tensor.matmul(w_ps[:], f_sb[:, k, d, :],
                                             a_tiles[k][:, d, sl],
                                             start=False, stop=(k == KT - 1))
                        nc.scalar.copy(w_sb[:, d, sl], w_ps[:])

                # complete the segment sum across cores
                cc_in = dram_pool.tile([NSEG, DEP, PW], BF16, tag="cc_in")
                cc_out = dram_pool.tile([NSEG, DEP, PW], BF16, tag="cc_out")
                nc.sync.dma_start(cc_in[:], w_sb[:])
                nc.gpsimd.collective_compute(
                    "AllReduce", mybir.AluOpType.add,
                    replica_groups=[list(range(NCORES))],
                    ins=[cc_in.opt()], outs=[cc_out.opt()],
                )
                mem_sb = mem_pool.tile([NSEG, DEP, PW], BF16)
                nc.sync.dma_start(mem_sb[:], cc_out[:])

                # phase B: per-item dot with its segment's memory row
                for k in range(KT):
                    for d in range(DEP):
                        g_ps = gps_pool.tile([128, PW], F32)
                        for h in range(2):
                            sl = slice(h * SC, (h + 1) * SC)
                            nc.tensor.matmul(g_ps[:, sl], oh_sb[:, k, :],
                                             mem_sb[:, d, sl],
                                             start=True, stop=True)
                        prod = prod_pool.tile([128, PW], BF16)
                        nc.vector.tensor_tensor(
                            out=prod[:], in0=a_tiles[k][:, d, :],
                            in1=g_ps[:], op=mybir.AluOpType.mult)
                        nc.scalar.activation(
                            out=prod[:], in_=prod[:],
                            func=mybir.ActivationFunctionType.Copy,
                            accum_out=r_all[:, k, d, i:i + 1])

            # epilogue: br = sum over pairs; ci = br / emb; crh = min_d ci
            br_sb = const.tile([128, KT, DEP], F32)
            nc.vector.tensor_reduce(out=br_sb[:], in_=r_all[:],
                                    axis=mybir.AxisListType.X,
                                    op=mybir.AluOpType.add)
            recip = const.tile([128, KT, DEP], F32)
            nc.vector.reciprocal(recip[:], emb_sb[:])
            ci_sb = const.tile([128, KT, DEP], F32)
            nc.vector.tensor_tensor(out=ci_sb[:], in0=br_sb[:], in1=recip[:],
                                    op=mybir.AluOpType.mult)
            crh_sb = const.tile([128, KT], F32)
            nc.vector.tensor_reduce(out=crh_sb[:], in_=ci_sb[:],
                                    axis=mybir.AxisListType.X,
                                    op=mybir.AluOpType.min)
            for k in range(KT):
                nc.sync.dma_start(br_d[k * 128:(k + 1) * 128, :], br_sb[:, k, :])
                nc.sync.dma_start(ci_d[k * 128:(k + 1) * 128, :], ci_sb[:, k, :])
                nc.sync.dma_start(crh_d[k * 128:(k + 1) * 128, :], crh_sb[:, k:k + 1])

    nc.compile()
    return nc


def _get_nc():
    global _CACHED
    if _CACHED is None:
        _CACHED = _build()
    return _CACHED


def kernel(batch_address, batch_embedding, batch_frequency, memory_matrix,
           segment_ids, _run_kwargs=None):
    addr = np.ascontiguousarray(np.asarray(batch_address, dtype=np.float32))
    emb = np.asarray(batch_embedding, dtype=np.float32)
    freq = np.asarray(batch_frequency, dtype=np.float32)
    mm = np.asarray(memory_matrix, dtype=np.float32)
    seg = np.asarray(segment_ids, dtype=np.int32)

    femb = emb * freq[:, None]                                   # [T, D]
    onehot = (seg[:, None] == np.arange(NSEG)[None, :]).astype(np.float32)
    F = femb[:, :, None] * onehot[:, None, :]                    # [T, D, B]
    eye = np.eye(NSEG, dtype=ml_dtypes.bfloat16)
    mm_chunks = np.ascontiguousarray(
        mm.reshape(NSEG, DEP, NPAIR, PW).transpose(2, 0, 1, 3)).astype(
            ml_dtypes.bfloat16)                                  # [5,16,5,1024]
    mm_zero = np.zeros_like(mm_chunks)

    in_maps = []
    for c in range(NCORES):
        sl = slice(c * TLOC, (c + 1) * TLOC)
        f_c = np.ascontiguousarray(
            F[sl].reshape(KT, 128, DEP, NSEG).transpose(1, 0, 2, 3)).astype(
                ml_dtypes.bfloat16)
        oh_c = np.ascontiguousarray(
            onehot[sl].reshape(KT, 128, NSEG).transpose(2, 0, 1)).astype(
                ml_dtypes.bfloat16)
        emb_c = np.ascontiguousarray(
            emb[sl].reshape(KT, 128, DEP).transpose(1, 0, 2))
        in_maps.append({
            "a": np.ascontiguousarray(addr[:, sl, :]).astype(ml_dtypes.bfloat16),
            "f": f_c,
            "oh": oh_c,
            "mm": mm_chunks if c == 0 else mm_zero,
            "eye": eye,
            "emb": emb_c,
        })

    nc = _get_nc()
    res = bass_utils.run_bass_kernel_spmd(
        nc, in_maps, core_ids=list(range(NCORES)), **(_run_kwargs or {}))

    br = np.concatenate([res.results[c]["br"] for c in range(NCORES)], axis=0)
    ci = np.concatenate([res.results[c]["ci"] for c in range(NCORES)], axis=0)
    crh = np.concatenate([res.results[c]["crh"] for c in range(NCORES)], axis=0)
    kernel.last_results = res
    return crh, ci, br


# revision 11
# speedup vs baseline: 1.2450x; 1.0524x over previous
"""Trainium2 Bass kernel for nn_MemoryModule_17145509445673 (scatter_memory).

Math (reference):
    f_emb   = batch_embedding * batch_frequency[:, None]              # [T, D]
    contrib = addr_t * f_emb[:, :, None]          (addr_t = [T, D, S])
    write   = segment_sum(contrib, segment_ids)                       # [B, D, S]
    mem     = memory_matrix + write
    basic_read[t, d] = sum_s addr[t, d, s] * mem[seg[t], d, s]
    cm_info = basic_read / batch_embedding ; cm_readhead = min_d cm_info

Strategy: shard the 4096 items across 8 cores (segment_ids are sorted, so
this is nearly segment-aligned).  Each core streams its 52 MB slice of
batch_address from HBM exactly once, in 5 slot-pairs of 1024 (2x512):
  - the A DMA casts fp32 -> bf16 in the SDMA datapath (SWDGE), so matmuls
    run at the PE's 1 cycle/row bf16 rate
  - PE computes the partial segment-sum w (lhsT = one-hot * f_emb),
    folding memory_matrix in via an identity matmul (nonzero on core 0 only)
  - an 8-core AllReduce completes w -> mem for the pair
  - PE re-expands mem per item (one-hot gather matmul), DVE multiplies the
    resident A tile into the gather PSUM tile in place, ACT row-reduces it
    into per-pair partial dots
"""

import sys

if "/opt/trn_rl_repo" not in sys.path:
    sys.path.insert(0, "/opt/trn_rl_repo")

import numpy as np
import ml_dtypes

from concourse import bacc, tile, mybir, bass_utils

DEP, TOTAL, SLOT, NSEG = 5, 4096, 5120, 16
NCORES = 8
TLOC = TOTAL // NCORES          # 512 items per core
KT = TLOC // 128                # 4 item tiles of 128
SC = 512                        # matmul slot granularity (PSUM bank)
PW = 2 * SC                     # 1024: phase-B pair width
NPAIR = SLOT // PW              # 5
F32 = mybir.dt.float32
BF16 = mybir.dt.bfloat16

_CACHED = None


def _build():
    nc = bacc.Bacc("TRN2", target_bir_lowering=False, debug=False,
                   enable_asserts=False, num_devices=NCORES)

    a_d = nc.dram_tensor("a", [DEP, TLOC, SLOT], BF16, kind="ExternalInput").ap()
    f_d = nc.dram_tensor("f", [128, KT, DEP, NSEG], BF16, kind="ExternalInput").ap()
    oh_d = nc.dram_tensor("oh", [NSEG, KT, 128], BF16, kind="ExternalInput").ap()
    mm_d = nc.dram_tensor("mm", [NPAIR, NSEG, DEP, PW], BF16, kind="ExternalInput").ap()
    eye_d = nc.dram_tensor("eye", [NSEG, NSEG], BF16, kind="ExternalInput").ap()
    emb_d = nc.dram_tensor("emb", [128, KT, DEP], F32, kind="ExternalInput").ap()

    br_d = nc.dram_tensor("br", [TLOC, DEP], F32, kind="ExternalOutput").ap()
    ci_d = nc.dram_tensor("ci", [TLOC, DEP], F32, kind="ExternalOutput").ap()
    crh_d = nc.dram_tensor("crh", [TLOC, 1], F32, kind="ExternalOutput").ap()

    with tile.TileContext(nc) as tc:
        with (
            tc.tile_pool(name="const", bufs=1) as const,
            tc.tile_pool(name="a_pool", bufs=11) as a_pool,
            tc.tile_pool(name="wsb", bufs=2) as wsb_pool,
            tc.tile_pool(name="mem", bufs=2) as mem_pool,
            tc.tile_pool(name="mmc", bufs=1) as mmc_pool,
            tc.tile_pool(name="prod", bufs=3) as prod_pool,
            tc.tile_pool(name="wps", bufs=2, space="PSUM") as wps_pool,
            tc.tile_pool(name="gps", bufs=3, space="PSUM") as gps_pool,
            tc.tile_pool(name="dram", bufs=4, space="DRAM") as dram_pool,
        ):
            f_sb = const.tile([128, KT, DEP, NSEG], BF16)
            oh_sb = const.tile([NSEG, KT, 128], BF16)
            eye_sb = const.tile([NSEG, NSEG], BF16)
            emb_sb = const.tile([128, KT, DEP], F32)
            r_all = const.tile([128, KT, DEP, NPAIR], F32)
            nc.sync.dma_start(f_sb[:], f_d[:])
            nc.sync.dma_start(oh_sb[:], oh_d[:])
            nc.sync.dma_start(eye_sb[:], eye_d[:])
            nc.sync.dma_start(emb_sb[:], emb_d[:])

            def load_pair(i):
                # A tiles: fp32 in HBM -> bf16 in SBUF (SWDGE cast), one DMA
                # per (item tile, half) so compute starts on the first half
                tiles = []
                for k in range(KT):
                    at = a_pool.tile([128, DEP, PW], BF16, tag="a_tile")
                    tiles.append(at)
                for k in range(KT):
                    src = a_d[:, k * 128:(k + 1) * 128, i * PW:(i + 1) * PW]
                    nc.sync.dma_start(tiles[k][:], src.rearrange("d p s -> p d s"))
                return tiles

            pending = {0: load_pair(0)}
            for i in range(NPAIR):
                # issue the NEXT pair's A loads before anything that depends
                # on this pair's collective, so the gpsimd DMA queue is not
                # head-of-line blocked behind CC(i)
                if i + 1 < NPAIR:
                    pending[i + 1] = load_pair(i + 1)
                a_tiles = pending.pop(i)

                mm_sb = mmc_pool.tile([NSEG, DEP, PW], BF16)
                nc.sync.dma_start(mm_sb[:], mm_d[i])

                # phase A: per-core partial segment sums (+ memory_matrix on core 0)
                w_sb = wsb_pool.tile([NSEG, DEP, PW], BF16)
                for d in range(DEP):
                    for h in range(2):
                        sl = slice(h * SC, (h + 1) * SC)
                        w_ps = wps_pool.tile([NSEG, SC], F32)
                        nc.tensor.matmul(w_ps[:], eye_sb[:], mm_sb[:, d, sl],
                                         start=True, stop=False)
                        for k in range(KT):
                            nc.tensor.matmul(w_ps[:], f_sb[:, k, d, :],
                                             a_tiles[k][:, d, sl],
                                             start=False, stop=(k == KT - 1))
                        nc.scalar.copy(w_sb[:, d, sl], w_ps[:])

                # complete the segment sum across cores
                cc_in = dram_pool.tile([NSEG, DEP, PW], BF16, tag="cc_in")
                cc_out = dram_pool.tile([NSEG, DEP, PW], BF16, tag="cc_out")
                nc.scalar.dma_start(cc_in[:], w_sb[:])
                nc.gpsimd.collective_compute(
                    "AllReduce", mybir.AluOpType.add,
                    replica_groups=[list(range(NCORES))],
                    ins=[cc_in.opt()], outs=[cc_out.opt()],
                )
                mem_sb = mem_pool.tile([NSEG, DEP, PW], BF16)
                nc.scalar.dma_start(mem_sb[:], cc_out[:])

                # phase B: per-item dot with its segment's memory row
                for k in range(KT):
                    for d in range(DEP):
                        g_ps = gps_pool.tile([128, PW], F32)
                        for h in range(2):
                            sl = slice(h * SC, (h + 1) * SC)
                            nc.tensor.matmul(g_ps[:, sl], oh_sb[:, k, :],
                                             mem_sb[:, d, sl],
                                             start=True, stop=True)
                        prod = prod_pool.tile([128, PW], BF16)
                        nc.vector.tensor_tensor(
                            out=prod[:], in0=a_tiles[k][:, d, :],
                            in1=g_ps[:], op=mybir.AluOpType.mult)
                        nc.scalar.activation(
                            out=prod[:], in_=prod[:],
                            func=mybir.ActivationFunctionType.Copy,
                            accum_out=r_all[:, k, d, i:i + 1])

            # epilogue: br = sum over pairs; ci = br / emb; crh = min_d ci
            br_sb = const.tile([128, KT, DEP], F32)
            nc.vector.tensor_reduce(out=br_sb[:], in_=r_all[:],
                                    axis=mybir.AxisListType.X,
                                    op=mybir.AluOpType.add)
            recip = const.tile([128, KT, DEP], F32)
            nc.vector.reciprocal(recip[:], emb_sb[:])
            ci_sb = const.tile([128, KT, DEP], F32)
            nc.vector.tensor_tensor(out=ci_sb[:], in0=br_sb[:], in1=recip[:],
                                    op=mybir.AluOpType.mult)
            crh_sb = const.tile([128, KT], F32)
            nc.vector.tensor_reduce(out=crh_sb[:], in_=ci_sb[:],
                                    axis=mybir.AxisListType.X,
                                    op=mybir.AluOpType.min)
            for k in range(KT):
                nc.sync.dma_start(br_d[k * 128:(k + 1) * 128, :], br_sb[:, k, :])
                nc.sync.dma_start(ci_d[k * 128:(k + 1) * 128, :], ci_sb[:, k, :])
                nc.sync.dma_start(crh_d[k * 128:(k + 1) * 128, :], crh_sb[:, k:k + 1])

    nc.compile()
    return nc


def _get_nc():
    global _CACHED
    if _CACHED is None:
        _CACHED = _build()
    return _CACHED


def kernel(batch_address, batch_embedding, batch_frequency, memory_matrix,
           segment_ids, _run_kwargs=None):
    addr = np.ascontiguousarray(np.asarray(batch_address, dtype=np.float32))
    emb = np.asarray(batch_embedding, dtype=np.float32)
    freq = np.asarray(batch_frequency, dtype=np.float32)
    mm = np.asarray(memory_matrix, dtype=np.float32)
    seg = np.asarray(segment_ids, dtype=np.int32)

    femb = emb * freq[:, None]                                   # [T, D]
    onehot = (seg[:, None] == np.arange(NSEG)[None, :]).astype(np.float32)
    F = femb[:, :, None] * onehot[:, None, :]                    # [T, D, B]
    eye = np.eye(NSEG, dtype=ml_dtypes.bfloat16)
    mm_chunks = np.ascontiguousarray(
        mm.reshape(NSEG, DEP, NPAIR, PW).transpose(2, 0, 1, 3)).astype(
            ml_dtypes.bfloat16)                                  # [5,16,5,1024]
    mm_zero = np.zeros_like(mm_chunks)

    in_maps = []
    for c in range(NCORES):
        sl = slice(c * TLOC, (c + 1) * TLOC)
        f_c = np.ascontiguousarray(
            F[sl].reshape(KT, 128, DEP, NSEG).transpose(1, 0, 2, 3)).astype(
                ml_dtypes.bfloat16)
        oh_c = np.ascontiguousarray(
            onehot[sl].reshape(KT, 128, NSEG).transpose(2, 0, 1)).astype(
                ml_dtypes.bfloat16)
        emb_c = np.ascontiguousarray(
            emb[sl].reshape(KT, 128, DEP).transpose(1, 0, 2))
        in_maps.append({
            "a": np.ascontiguousarray(addr[:, sl, :]).astype(ml_dtypes.bfloat16),
            "f": f_c,
            "oh": oh_c,
            "mm": mm_chunks if c == 0 else mm_zero,
            "eye": eye,
            "emb": emb_c,
        })

    nc = _get_nc()
    res = bass_utils.run_bass_kernel_spmd(
        nc, in_maps, core_ids=list(range(NCORES)), **(_run_kwargs or {}))

    br = np.concatenate([res.results[c]["br"] for c in range(NCORES)], axis=0)
    ci = np.concatenate([res.results[c]["ci"] for c in range(NCORES)], axis=0)
    crh = np.concatenate([res.results[c]["crh"] for c in range(NCORES)], axis=0)
    kernel.last_results = res
    return crh, ci, br


# revision 12
# speedup vs baseline: 1.4130x; 1.1349x over previous
"""Trainium2 Bass kernel for nn_MemoryModule_17145509445673 (scatter_memory).

Math (reference):
    f_emb   = batch_embedding * batch_frequency[:, None]              # [T, D]
    contrib = addr_t * f_emb[:, :, None]          (addr_t = [T, D, S])
    write   = segment_sum(contrib, segment_ids)                       # [B, D, S]
    mem     = memory_matrix + write
    basic_read[t, d] = sum_s addr[t, d, s] * mem[seg[t], d, s]
    cm_info = basic_read / batch_embedding ; cm_readhead = min_d cm_info

Strategy: shard the 4096 items across 8 cores (segment_ids are sorted, so
this is nearly segment-aligned).  Each core streams its 52 MB slice of
batch_address from HBM exactly once, in 5 slot-pairs of 1024 (2x512):
  - the A DMA casts fp32 -> bf16 in the SDMA datapath (SWDGE), so matmuls
    run at the PE's 1 cycle/row bf16 rate
  - PE computes the partial segment-sum w (lhsT = one-hot * f_emb),
    folding memory_matrix in via an identity matmul (nonzero on core 0 only)
  - an 8-core AllReduce completes w -> mem for the pair
  - PE re-expands mem per item (one-hot gather matmul), DVE multiplies the
    resident A tile into the gather PSUM tile in place, ACT row-reduces it
    into per-pair partial dots
"""

import sys

if "/opt/trn_rl_repo" not in sys.path:
    sys.path.insert(0, "/opt/trn_rl_repo")

import numpy as np
import ml_dtypes

from concourse import bacc, tile, mybir, bass_utils

DEP, TOTAL, SLOT, NSEG = 5, 4096, 5120, 16
NCORES = 8
TLOC = TOTAL // NCORES          # 512 items per core
KT = TLOC // 128                # 4 item tiles of 128
SC = 512                        # matmul slot granularity (PSUM bank)
PW = 2 * SC                     # 1024: phase-B pair width
NPAIR = SLOT // PW              # 5
F32 = mybir.dt.float32
BF16 = mybir.dt.bfloat16

_CACHED = None


def _build():
    nc = bacc.Bacc("TRN2", target_bir_lowering=False, debug=False,
                   enable_asserts=False, num_devices=NCORES)

    a_d = nc.dram_tensor("a", [DEP, TLOC, SLOT], BF16, kind="ExternalInput").ap()
    f_d = nc.dram_tensor("f", [128, KT, DEP, NSEG], BF16, kind="ExternalInput").ap()
    oh_d = nc.dram_tensor("oh", [NSEG, KT, 128], BF16, kind="ExternalInput").ap()
    mm_d = nc.dram_tensor("mm", [NPAIR, NSEG, DEP, PW], BF16, kind="ExternalInput").ap()
    eye_d = nc.dram_tensor("eye", [NSEG, NSEG], BF16, kind="ExternalInput").ap()
    emb_d = nc.dram_tensor("emb", [128, KT, DEP], F32, kind="ExternalInput").ap()

    br_d = nc.dram_tensor("br", [TLOC, DEP], F32, kind="ExternalOutput").ap()
    ci_d = nc.dram_tensor("ci", [TLOC, DEP], F32, kind="ExternalOutput").ap()
    crh_d = nc.dram_tensor("crh", [TLOC, 1], F32, kind="ExternalOutput").ap()

    with tile.TileContext(nc) as tc:
        with (
            tc.tile_pool(name="const", bufs=1) as const,
            tc.tile_pool(name="a_pool", bufs=11) as a_pool,
            tc.tile_pool(name="wsb", bufs=2) as wsb_pool,
            tc.tile_pool(name="mem", bufs=2) as mem_pool,
            tc.tile_pool(name="mmc", bufs=1) as mmc_pool,
            tc.tile_pool(name="prod", bufs=3) as prod_pool,
            tc.tile_pool(name="wps", bufs=2, space="PSUM") as wps_pool,
            tc.tile_pool(name="gps", bufs=3, space="PSUM") as gps_pool,
            tc.tile_pool(name="dram", bufs=4, space="DRAM") as dram_pool,
        ):
            f_sb = const.tile([128, KT, DEP, NSEG], BF16)
            oh_sb = const.tile([NSEG, KT, 128], BF16)
            eye_sb = const.tile([NSEG, NSEG], BF16)
            emb_sb = const.tile([128, KT, DEP], F32)
            r_all = const.tile([128, KT, DEP, NPAIR], F32)
            nc.sync.dma_start(f_sb[:], f_d[:])
            nc.sync.dma_start(oh_sb[:], oh_d[:])
            nc.sync.dma_start(eye_sb[:], eye_d[:])
            nc.sync.dma_start(emb_sb[:], emb_d[:])

            def load_pair(i):
                # A tiles: fp32 in HBM -> bf16 in SBUF (SWDGE cast), one DMA
                # per (item tile, half) so compute starts on the first half
                tiles = []
                for k in range(KT):
                    at = a_pool.tile([128, DEP, PW], BF16, tag="a_tile")
                    tiles.append(at)
                for k in range(KT):
                    src = a_d[:, k * 128:(k + 1) * 128, i * PW:(i + 1) * PW]
                    nc.sync.dma_start(tiles[k][:], src.rearrange("d p s -> p d s"))
                return tiles

            # ---- sweep 1: stream A, phase A segment sums, fire CCs ----
            cc_outs = []
            for i in range(NPAIR):
                a_tiles = load_pair(i)

                mm_sb = mmc_pool.tile([NSEG, DEP, PW], BF16)
                nc.sync.dma_start(mm_sb[:], mm_d[i])

                w_sb = wsb_pool.tile([NSEG, DEP, PW], BF16)
                for d in range(DEP):
                    for h in range(2):
                        sl = slice(h * SC, (h + 1) * SC)
                        w_ps = wps_pool.tile([NSEG, SC], F32)
                        nc.tensor.matmul(w_ps[:], eye_sb[:], mm_sb[:, d, sl],
                                         start=True, stop=False)
                        for k in range(KT):
                            nc.tensor.matmul(w_ps[:], f_sb[:, k, d, :],
                                             a_tiles[k][:, d, sl],
                                             start=False, stop=(k == KT - 1))
                        nc.scalar.copy(w_sb[:, d, sl], w_ps[:])

                cc_in = dram_pool.tile([NSEG, DEP, PW], BF16, tag="cc_in")
                cc_out = dram_pool.tile([NSEG, DEP, PW], BF16, tag="cc_out",
                                        bufs=NPAIR)
                nc.scalar.dma_start(cc_in[:], w_sb[:])
                nc.gpsimd.collective_compute(
                    "AllReduce", mybir.AluOpType.add,
                    replica_groups=[list(range(NCORES))],
                    ins=[cc_in.opt()], outs=[cc_out.opt()],
                )
                cc_outs.append(cc_out)

            # ---- sweep 2: re-stream A, gather + dot ----
            for i in range(NPAIR):
                a_tiles = load_pair(i)

                mem_sb = mem_pool.tile([NSEG, DEP, PW], BF16)
                nc.scalar.dma_start(mem_sb[:], cc_outs[i][:])

                for k in range(KT):
                    for d in range(DEP):
                        g_ps = gps_pool.tile([128, PW], F32)
                        for h in range(2):
                            sl = slice(h * SC, (h + 1) * SC)
                            nc.tensor.matmul(g_ps[:, sl], oh_sb[:, k, :],
                                             mem_sb[:, d, sl],
                                             start=True, stop=True)
                        prod = prod_pool.tile([128, PW], BF16)
                        nc.vector.tensor_tensor(
                            out=prod[:], in0=a_tiles[k][:, d, :],
                            in1=g_ps[:], op=mybir.AluOpType.mult)
                        nc.scalar.activation(
                            out=prod[:], in_=prod[:],
                            func=mybir.ActivationFunctionType.Copy,
                            accum_out=r_all[:, k, d, i:i + 1])

            # epilogue: br = sum over pairs; ci = br / emb; crh = min_d ci
            br_sb = const.tile([128, KT, DEP], F32)
            nc.vector.tensor_reduce(out=br_sb[:], in_=r_all[:],
                                    axis=mybir.AxisListType.X,
                                    op=mybir.AluOpType.add)
            recip = const.tile([128, KT, DEP], F32)
            nc.vector.reciprocal(recip[:], emb_sb[:])
            ci_sb = const.tile([128, KT, DEP], F32)
            nc.vector.tensor_tensor(out=ci_sb[:], in0=br_sb[:], in1=recip[:],
                                    op=mybir.AluOpType.mult)
            crh_sb = const.tile([128, KT], F32)
            nc.vector.tensor_reduce(out=crh_sb[:], in_=ci_sb[:],
                                    axis=mybir.AxisListType.X,
                                    op=mybir.AluOpType.min)
            for k in range(KT):
                nc.sync.dma_start(br_d[k * 128:(k + 1) * 128, :], br_sb[:, k, :])
                nc.sync.dma_start(ci_d[k * 128:(k + 1) * 128, :], ci_sb[:, k, :])
                nc.sync.dma_start(crh_d[k * 128:(k + 1) * 128, :], crh_sb[:, k:k + 1])

    nc.compile()
    return nc


def _get_nc():
    global _CACHED
    if _CACHED is None:
        _CACHED = _build()
    return _CACHED


def kernel(batch_address, batch_embedding, batch_frequency, memory_matrix,
           segment_ids, _run_kwargs=None):
    addr = np.ascontiguousarray(np.asarray(batch_address, dtype=np.float32))
    emb = np.asarray(batch_embedding, dtype=np.float32)
    freq = np.asarray(batch_frequency, dtype=np.float32)
    mm = np.asarray(memory_matrix, dtype=np.float32)
    seg = np.asarray(segment_ids, dtype=np.int32)

    femb = emb * freq[:, None]                                   # [T, D]
    onehot = (seg[:, None] == np.arange(NSEG)[None, :]).astype(np.float32)
    F = femb[:, :, None] * onehot[:, None, :]                    # [T, D, B]
    eye = np.eye(NSEG, dtype=ml_dtypes.bfloat16)
    mm_chunks = np.ascontiguousarray(
        mm.reshape(NSEG, DEP, NPAIR, PW).transpose(2, 0, 1, 3)).astype(
            ml_dtypes.bfloat16)                                  # [5,16,5,1024]
    mm_zero = np.zeros_like(mm_chunks)

    in_maps = []
    for c in range(NCORES):
        sl = slice(c * TLOC, (c + 1) * TLOC)
        f_c = np.ascontiguousarray(
            F[sl].reshape(KT, 128, DEP, NSEG).transpose(1, 0, 2, 3)).astype(
                ml_dtypes.bfloat16)
        oh_c = np.ascontiguousarray(
            onehot[sl].reshape(KT, 128, NSEG).transpose(2, 0, 1)).astype(
                ml_dtypes.bfloat16)
        emb_c = np.ascontiguousarray(
            emb[sl].reshape(KT, 128, DEP).transpose(1, 0, 2))
        in_maps.append({
            "a": np.ascontiguousarray(addr[:, sl, :]).astype(ml_dtypes.bfloat16),
            "f": f_c,
            "oh": oh_c,
            "mm": mm_chunks if c == 0 else mm_zero,
            "eye": eye,
            "emb": emb_c,
        })

    nc = _get_nc()
    res = bass_utils.run_bass_kernel_spmd(
        nc, in_maps, core_ids=list(range(NCORES)), **(_run_kwargs or {}))

    br = np.concatenate([res.results[c]["br"] for c in range(NCORES)], axis=0)
    ci = np.concatenate([res.results[c]["ci"] for c in range(NCORES)], axis=0)
    crh = np.concatenate([res.results[c]["crh"] for c in range(NCORES)], axis=0)
    kernel.last_results = res
    return crh, ci, br


# revision 13
# speedup vs baseline: 1.5285x; 1.0817x over previous
"""Trainium2 Bass kernel for nn_MemoryModule_17145509445673 (scatter_memory).

Math (reference):
    f_emb   = batch_embedding * batch_frequency[:, None]              # [T, D]
    contrib = addr_t * f_emb[:, :, None]          (addr_t = [T, D, S])
    write   = segment_sum(contrib, segment_ids)                       # [B, D, S]
    mem     = memory_matrix + write
    basic_read[t, d] = sum_s addr[t, d, s] * mem[seg[t], d, s]
    cm_info = basic_read / batch_embedding ; cm_readhead = min_d cm_info

Strategy: shard the 4096 items across 8 cores (segment_ids are sorted, so
this is nearly segment-aligned).  Each core streams its 52 MB slice of
batch_address from HBM exactly once, in 5 slot-pairs of 1024 (2x512):
  - the A DMA casts fp32 -> bf16 in the SDMA datapath (SWDGE), so matmuls
    run at the PE's 1 cycle/row bf16 rate
  - PE computes the partial segment-sum w (lhsT = one-hot * f_emb),
    folding memory_matrix in via an identity matmul (nonzero on core 0 only)
  - an 8-core AllReduce completes w -> mem for the pair
  - PE re-expands mem per item (one-hot gather matmul), DVE multiplies the
    resident A tile into the gather PSUM tile in place, ACT row-reduces it
    into per-pair partial dots
"""

import sys

if "/opt/trn_rl_repo" not in sys.path:
    sys.path.insert(0, "/opt/trn_rl_repo")

import numpy as np
import ml_dtypes

from concourse import bacc, tile, mybir, bass_utils

DEP, TOTAL, SLOT, NSEG = 5, 4096, 5120, 16
NCORES = 8
TLOC = TOTAL // NCORES          # 512 items per core
KT = TLOC // 128                # 4 item tiles of 128
SC = 512                        # matmul slot granularity (PSUM bank)
PW = 2 * SC                     # 1024: phase-B pair width
NPAIR = SLOT // PW              # 5
F32 = mybir.dt.float32
BF16 = mybir.dt.bfloat16

_CACHED = None


def _build():
    nc = bacc.Bacc("TRN2", target_bir_lowering=False, debug=False,
                   enable_asserts=False, num_devices=NCORES)

    a_d = nc.dram_tensor("a", [DEP, TLOC, SLOT], BF16, kind="ExternalInput").ap()
    f_d = nc.dram_tensor("f", [128, KT, DEP, NSEG], BF16, kind="ExternalInput").ap()
    oh_d = nc.dram_tensor("oh", [2 * NSEG, KT, 128], BF16, kind="ExternalInput").ap()
    mm_d = nc.dram_tensor("mm", [NPAIR, NSEG, DEP, PW], BF16, kind="ExternalInput").ap()
    emb_d = nc.dram_tensor("emb", [128, KT, DEP], F32, kind="ExternalInput").ap()

    br_d = nc.dram_tensor("br", [TLOC, DEP], F32, kind="ExternalOutput").ap()
    ci_d = nc.dram_tensor("ci", [TLOC, DEP], F32, kind="ExternalOutput").ap()
    crh_d = nc.dram_tensor("crh", [TLOC, 1], F32, kind="ExternalOutput").ap()

    with tile.TileContext(nc) as tc:
        with (
            tc.tile_pool(name="const", bufs=1) as const,
            tc.tile_pool(name="a_pool", bufs=7) as a_pool,
            tc.tile_pool(name="a_res", bufs=KT) as a_res_pool,
            tc.tile_pool(name="wsb", bufs=2) as wsb_pool,
            tc.tile_pool(name="mem", bufs=2) as mem_pool,
            tc.tile_pool(name="prod", bufs=3) as prod_pool,
            tc.tile_pool(name="wps", bufs=2, space="PSUM") as wps_pool,
            tc.tile_pool(name="gps", bufs=3, space="PSUM") as gps_pool,
            tc.tile_pool(name="dram", bufs=4, space="DRAM") as dram_pool,
        ):
            f_sb = const.tile([128, KT, DEP, NSEG], BF16)
            oh_sb = const.tile([2 * NSEG, KT, 128], BF16)
            emb_sb = const.tile([128, KT, DEP], F32)
            r_all = const.tile([128, KT, DEP, NPAIR], F32)
            nc.sync.dma_start(f_sb[:], f_d[:])
            nc.sync.dma_start(oh_sb[:], oh_d[:])
            nc.sync.dma_start(emb_sb[:], emb_d[:])

            def load_pair(i, pool):
                tiles = []
                for k in range(KT):
                    at = pool.tile([128, DEP, PW], BF16, tag="a_tile")
                    tiles.append(at)
                for k in range(KT):
                    src = a_d[:, k * 128:(k + 1) * 128, i * PW:(i + 1) * PW]
                    nc.sync.dma_start(tiles[k][:], src.rearrange("d p s -> p d s"))
                return tiles

            # ---- sweep 1: stream A, phase A segment sums, fire batched CCs ----
            CC_BATCHES = [(0, 1), (2, 3), (4,)]
            batch_of = {i: b for b, pairs in enumerate(CC_BATCHES) for i in pairs}
            cc_ins, cc_outs = [], []
            for pairs in CC_BATCHES:
                n = len(pairs)
                cin = dram_pool.tile([n, NSEG, DEP, PW], BF16, tag="cc_in",
                                     bufs=len(CC_BATCHES))
                cout = dram_pool.tile([n, NSEG, DEP, PW], BF16, tag="cc_out",
                                      bufs=len(CC_BATCHES))
                cc_ins.append(cin)
                cc_outs.append(cout)

            resident = {}
            for i in range(NPAIR):
                a_tiles = load_pair(i, a_res_pool if i == 0 else a_pool)
                if i == 0:
                    resident[i] = a_tiles

                w_sb = wsb_pool.tile([NSEG, DEP, PW], BF16)
                for d in range(DEP):
                    for h in range(2):
                        sl = slice(h * SC, (h + 1) * SC)
                        w_ps = wps_pool.tile([NSEG, SC], F32)
                        for k in range(KT):
                            nc.tensor.matmul(w_ps[:], f_sb[:, k, d, :],
                                             a_tiles[k][:, d, sl],
                                             start=(k == 0), stop=(k == KT - 1))
                        nc.scalar.copy(w_sb[:, d, sl], w_ps[:])

                b = batch_of[i]
                slot = CC_BATCHES[b].index(i)
                nc.scalar.dma_start(cc_ins[b][slot], w_sb[:])
                if i == CC_BATCHES[b][-1]:
                    nc.gpsimd.collective_compute(
                        "AllReduce", mybir.AluOpType.add,
                        replica_groups=[list(range(NCORES))],
                        ins=[cc_ins[b].opt()], outs=[cc_outs[b].opt()],
                    )

            # ---- sweep 2: re-stream A (pair 0 resident), gather(mem+memmat) + dot ----
            for i in range(NPAIR):
                a_tiles = resident.get(i) or load_pair(i, a_pool)

                # gather rhs: rows 0:16 = allreduced write sums, 16:32 = memory
                # matrix chunk; the stacked one-hot sums them during the gather
                grhs = mem_pool.tile([2 * NSEG, DEP, PW], BF16)
                b = batch_of[i]
                slot = CC_BATCHES[b].index(i)
                nc.scalar.dma_start(grhs[:NSEG], cc_outs[b][slot])
                nc.sync.dma_start(grhs[NSEG:], mm_d[i])

                for k in range(KT):
                    for d in range(DEP):
                        g_ps = gps_pool.tile([128, PW], F32)
                        for h in range(2):
                            sl = slice(h * SC, (h + 1) * SC)
                            nc.tensor.matmul(g_ps[:, sl], oh_sb[:, k, :],
                                             grhs[:, d, sl],
                                             start=True, stop=True)
                        prod = prod_pool.tile([128, PW], BF16)
                        nc.vector.tensor_tensor(
                            out=prod[:], in0=a_tiles[k][:, d, :],
                            in1=g_ps[:], op=mybir.AluOpType.mult)
                        nc.scalar.activation(
                            out=prod[:], in_=prod[:],
                            func=mybir.ActivationFunctionType.Copy,
                            accum_out=r_all[:, k, d, i:i + 1])

            # epilogue: br = sum over pairs; ci = br / emb; crh = min_d ci
            br_sb = const.tile([128, KT, DEP], F32)
            nc.vector.tensor_reduce(out=br_sb[:], in_=r_all[:],
                                    axis=mybir.AxisListType.X,
                                    op=mybir.AluOpType.add)
            recip = const.tile([128, KT, DEP], F32)
            nc.vector.reciprocal(recip[:], emb_sb[:])
            ci_sb = const.tile([128, KT, DEP], F32)
            nc.vector.tensor_tensor(out=ci_sb[:], in0=br_sb[:], in1=recip[:],
                                    op=mybir.AluOpType.mult)
            crh_sb = const.tile([128, KT], F32)
            nc.vector.tensor_reduce(out=crh_sb[:], in_=ci_sb[:],
                                    axis=mybir.AxisListType.X,
                                    op=mybir.AluOpType.min)
            for k in range(KT):
                nc.sync.dma_start(br_d[k * 128:(k + 1) * 128, :], br_sb[:, k, :])
                nc.sync.dma_start(ci_d[k * 128:(k + 1) * 128, :], ci_sb[:, k, :])
                nc.sync.dma_start(crh_d[k * 128:(k + 1) * 128, :], crh_sb[:, k:k + 1])

    nc.compile()
    return nc


def _get_nc():
    global _CACHED
    if _CACHED is None:
        _CACHED = _build()
    return _CACHED


def kernel(batch_address, batch_embedding, batch_frequency, memory_matrix,
           segment_ids, _run_kwargs=None):
    addr = np.ascontiguousarray(np.asarray(batch_address, dtype=np.float32))
    emb = np.asarray(batch_embedding, dtype=np.float32)
    freq = np.asarray(batch_frequency, dtype=np.float32)
    mm = np.asarray(memory_matrix, dtype=np.float32)
    seg = np.asarray(segment_ids, dtype=np.int32)

    femb = emb * freq[:, None]                                   # [T, D]
    onehot = (seg[:, None] == np.arange(NSEG)[None, :]).astype(np.float32)
    F = femb[:, :, None] * onehot[:, None, :]                    # [T, D, B]
    mm_chunks = np.ascontiguousarray(
        mm.reshape(NSEG, DEP, NPAIR, PW).transpose(2, 0, 1, 3)).astype(
            ml_dtypes.bfloat16)                                  # [5,16,5,1024]

    in_maps = []
    for c in range(NCORES):
        sl = slice(c * TLOC, (c + 1) * TLOC)
        f_c = np.ascontiguousarray(
            F[sl].reshape(KT, 128, DEP, NSEG).transpose(1, 0, 2, 3)).astype(
                ml_dtypes.bfloat16)
        oh1 = onehot[sl].reshape(KT, 128, NSEG).transpose(2, 0, 1)
        oh_c = np.ascontiguousarray(
            np.concatenate([oh1, oh1], axis=0)).astype(ml_dtypes.bfloat16)
        emb_c = np.ascontiguousarray(
            emb[sl].reshape(KT, 128, DEP).transpose(1, 0, 2))
        in_maps.append({
            "a": np.ascontiguousarray(addr[:, sl, :]).astype(ml_dtypes.bfloat16),
            "f": f_c,
            "oh": oh_c,
            "mm": mm_chunks,
            "emb": emb_c,
        })

    nc = _get_nc()
    res = bass_utils.run_bass_kernel_spmd(
        nc, in_maps, core_ids=list(range(NCORES)), **(_run_kwargs or {}))

    br = np.concatenate([res.results[c]["br"] for c in range(NCORES)], axis=0)
    ci = np.concatenate([res.results[c]["ci"] for c in range(NCORES)], axis=0)
    crh = np.concatenate([res.results[c]["crh"] for c in range(NCORES)], axis=0)
    kernel.last_results = res
    return crh, ci, br
